# revision 6
# baseline (speedup 1.0000x reference)
"""HadLinear Trainium2 kernel: out = blockwise_FWHT(x)/sqrt(1024) @ w.T.

Strategy (8 NeuronCores, tensor-parallel over output features):
  - The blockwise Hadamard is linear: out = x @ (B @ w.T) where
    B = blockdiag(H_1024, x4) / 32 is symmetric.  Stage 1 computes
    V = B @ w.T on-device using the Kronecker split H_1024 = H_8 (x) H_128:
      * the H_8 factor mixes the 8 contraction chunks of each block —
        3 butterfly stages of add/sub on the Vector engine (cheap);
      * the H_128 factor is one matmul per chunk with a constant
        +-1/32 lhsT (exact in bf16) — 32 matmuls instead of 256.
  - w is column-sharded: core c owns output features [c*512, (c+1)*512).
    Every core streams the full x (tokens-major tiles, host-transposed
    to feature-major layout) and computes out[:, c*512:(c+1)*512].
  - Matmuls run in bf16 with fp32 PSUM accumulation.
"""

import numpy as np
import ml_dtypes

import concourse.bacc as bacc
import concourse.tile as tile
import concourse.mybir as mybir
from concourse.bass_utils import run_bass_kernel_spmd

N_CORES = 8
B, S, D = 4, 2048, 4096          # input (B, S, D)
TOK = B * S                      # 8192 tokens
BLOCK = 1024                     # Hadamard block
OUT_PER_CORE = D // N_CORES      # 512 output features per core
M_CHUNKS = TOK // 128            # 64 token chunks
K_CHUNKS = D // 128              # 32 contraction chunks
QR = BLOCK // 128                # 8 chunks per Hadamard block
NBLK = D // BLOCK                # 4 Hadamard blocks

BF16 = ml_dtypes.bfloat16

_PROGRAM = None


def _h128_table():
    """h[p, q] = H_128[p, q] / 32, bf16 (exact: +-2^-5)."""
    idx = np.arange(128)
    anded = idx[:, None] & idx[None, :]
    par = np.zeros_like(anded)
    v = anded
    while v.any():
        par ^= v & 1
        v >>= 1
    return ((1 - 2 * par).astype(np.float32) / 32.0).astype(BF16)


def _build_program(reps=1, m_chunks=M_CHUNKS):
    nc = bacc.Bacc("TRN2", target_bir_lowering=False, debug=False,
                   num_devices=N_CORES)
    # A[m, p, k, t] = x[m*128 + t, k*128 + p]  (feature-major token tiles)
    x_d = nc.dram_tensor("xa", [m_chunks, 128, K_CHUNKS, 128],
                         mybir.dt.bfloat16, kind="ExternalInput")
    # wt[p, b, j2, j1, j0, o] = w[c*512 + o, (b*8 + j2*4 + j1*2 + j0)*128 + p]
    w_d = nc.dram_tensor("wt", [128, NBLK, 2, 2, 2, OUT_PER_CORE],
                         mybir.dt.bfloat16, kind="ExternalInput")
    h_d = nc.dram_tensor("h128", [128, 128],
                         mybir.dt.bfloat16, kind="ExternalInput")
    # out[m, t, o] = out_full[m*128 + t, c*512 + o]
    o_d = nc.dram_tensor("out", [m_chunks, 128, OUT_PER_CORE],
                         mybir.dt.float32, kind="ExternalOutput")

    PRE = 2  # m-chunks whose GEMM overlaps stage 1 (block-wise partials)

    with tile.TileContext(nc) as tc:
        with (
            tc.tile_pool(name="consts", bufs=1) as consts,
            tc.tile_pool(name="wstage", bufs=1) as wstage,
            tc.tile_pool(name="xin", bufs=6) as xin,
            tc.tile_pool(name="ostage", bufs=4) as ostage,
            tc.tile_pool(name="ps1", bufs=4, space="PSUM") as ps1,
            tc.tile_pool(name="psp", bufs=1, space="PSUM") as psp,
            tc.tile_pool(name="ps2", bufs=2, space="PSUM") as ps2,
        ):
            hm = consts.tile([128, 128], mybir.dt.bfloat16)
            wt = wstage.tile([128, NBLK, 2, 2, 2, OUT_PER_CORE],
                             mybir.dt.bfloat16)
            ub = wstage.tile([128, NBLK, 2, 2, 2, OUT_PER_CORE],
                             mybir.dt.bfloat16)
            wp = wstage.tile([128, K_CHUNKS, OUT_PER_CORE], mybir.dt.bfloat16)
            pre_acc = [
                psp.tile([128, OUT_PER_CORE], mybir.dt.float32,
                         name=f"pracc{i}")
                for i in range(PRE)
            ]
            nc.sync.dma_start(hm[:], h_d[:])

            for rep in range(reps):
                # DMA order: wt blocks and the PRE x-tiles interleaved so
                # stage 1 starts immediately and partials never starve.
                pre_xt = []
                for b in range(NBLK):
                    nc.sync.dma_start(wt[:, b], w_d[:, b])
                    if b < PRE:
                        xt = xin.tile([128, K_CHUNKS, 128], mybir.dt.bfloat16)
                        nc.sync.dma_start(xt[:], x_d[b])
                        pre_xt.append(xt)

                # ---- Stage 1: V = B @ w.T via H_8 (x) H_128, with the
                # first PRE m-chunks' GEMM interleaved per block ----
                for b in range(NBLK):
                    # H_8 butterflies across the 8 chunks of block b
                    # (bit0, then bit1, then bit2), ping-pong wt <-> ub.
                    nc.vector.tensor_add(
                        out=ub[:, b, :, :, 0, :],
                        in0=wt[:, b, :, :, 0, :], in1=wt[:, b, :, :, 1, :])
                    nc.vector.tensor_sub(
                        out=ub[:, b, :, :, 1, :],
                        in0=wt[:, b, :, :, 0, :], in1=wt[:, b, :, :, 1, :])
                    nc.vector.tensor_add(
                        out=wt[:, b, :, 0, :, :],
                        in0=ub[:, b, :, 0, :, :], in1=ub[:, b, :, 1, :, :])
                    nc.vector.tensor_sub(
                        out=wt[:, b, :, 1, :, :],
                        in0=ub[:, b, :, 0, :, :], in1=ub[:, b, :, 1, :, :])
                    nc.vector.tensor_add(
                        out=ub[:, b, 0, :, :, :],
                        in0=wt[:, b, 0, :, :, :], in1=wt[:, b, 1, :, :, :])
                    nc.vector.tensor_sub(
                        out=ub[:, b, 1, :, :, :],
                        in0=wt[:, b, 0, :, :, :], in1=wt[:, b, 1, :, :, :])
                    # H_128 factor: one matmul per chunk, lhsT = H_128/32
                    for j in range(QR):
                        kq = b * QR + j
                        acc = ps1.tile([128, OUT_PER_CORE], mybir.dt.float32)
                        nc.tensor.matmul(
                            acc[:],
                            hm[:],
                            ub[:, b, j // 4, (j // 2) % 2, j % 2, :],
                            start=True, stop=True,
                        )
                        if j % 2 == 0:
                            nc.scalar.copy(out=wp[:, kq, :], in_=acc[:])
                        else:
                            nc.vector.tensor_copy(out=wp[:, kq, :], in_=acc[:])
                    # partial GEMM for the PRE chunks over this block's kqs
                    for i in range(PRE):
                        for j in range(QR):
                            kq = b * QR + j
                            nc.tensor.matmul(
                                pre_acc[i][:],
                                pre_xt[i][:, kq, :],
                                wp[:, kq, :],
                                start=(b == 0 and j == 0),
                                stop=(b == NBLK - 1 and j == QR - 1),
                            )
                for i in range(PRE):
                    ot = ostage.tile([128, OUT_PER_CORE], mybir.dt.float32)
                    nc.vector.tensor_copy(out=ot[:], in_=pre_acc[i][:])
                    nc.sync.dma_start(o_d[i], ot[:])

                # ---- Stage 2: out[m] = X[m] @ V ----
                for m in range(PRE, m_chunks):
                    xt = xin.tile([128, K_CHUNKS, 128], mybir.dt.bfloat16)
                    nc.sync.dma_start(xt[:], x_d[m])
                    acc = ps2.tile([128, OUT_PER_CORE], mybir.dt.float32)
                    for k in range(K_CHUNKS):
                        nc.tensor.matmul(
                            acc[:],
                            xt[:, k, :],
                            wp[:, k, :],
                            start=(k == 0),
                            stop=(k == K_CHUNKS - 1),
                        )
                    ot = ostage.tile([128, OUT_PER_CORE], mybir.dt.float32)
                    nc.vector.tensor_copy(out=ot[:], in_=acc[:])
                    nc.sync.dma_start(o_d[m], ot[:])

    nc.compile()
    return nc


def _get_program():
    global _PROGRAM
    if _PROGRAM is None:
        _PROGRAM = _build_program()
    return _PROGRAM


def _prep_inputs(input, weight):
    x = np.asarray(input, dtype=np.float32).reshape(TOK, D)
    w = np.asarray(weight, dtype=np.float32)
    # A[m, p, k, t] = x[m*128+t, k*128+p]
    xa = np.ascontiguousarray(
        x.reshape(M_CHUNKS, 128, K_CHUNKS, 128).transpose(0, 3, 2, 1)
    ).astype(BF16)
    hm = _h128_table()
    in_maps = []
    for c in range(N_CORES):
        wsl = w[c * OUT_PER_CORE:(c + 1) * OUT_PER_CORE, :]  # [512, 4096]
        wt = np.ascontiguousarray(
            wsl.T.reshape(K_CHUNKS, 128, OUT_PER_CORE).transpose(1, 0, 2)
        ).astype(BF16).reshape(128, NBLK, 2, 2, 2, OUT_PER_CORE)
        in_maps.append({"xa": xa, "wt": wt, "h128": hm})
    return in_maps


def kernel(input, weight):
    import time as _time

    nc = _get_program()
    in_maps = _prep_inputs(input, weight)
    # The axon-side XLA compile of the bass_exec custom call is
    # intermittently flaky (CallFunctionObjArgs INTERNAL error) on first
    # compile in a fresh process; a clean retry re-lowers and succeeds.
    last_exc = None
    for attempt in range(3):
        try:
            res = run_bass_kernel_spmd(nc, in_maps, list(range(N_CORES)))
            break
        except Exception as exc:  # noqa: BLE001 - retry transient compile/exec
            # Also rides out a stale device wedge (NRT_EXEC_UNIT_UNRECOVERABLE),
            # which clears on a ~1-2 minute timescale.
            last_exc = exc
            _time.sleep(30.0 * (attempt + 1))
    else:
        raise last_exc
    parts = [res.results[c]["out"].reshape(TOK, OUT_PER_CORE)
             for c in range(N_CORES)]
    out = np.concatenate(parts, axis=1).reshape(B, S, D)
    return np.ascontiguousarray(out, dtype=np.float32)
